# revision 43
# baseline (speedup 1.0000x reference)
"""Trainium2 Bass kernel for nn_BACMoE_Model (moe_routing).

Self-contained. Strategy (8 NeuronCores, SPMD): core i = (b = i//4,
row-quarter q = i%4). Each core computes its 26-row output window (+halo):
routers (replicated), high-pass FFT filter via 41-frequency-restricted DFT
matmuls, 10 expert branches (BN/1x1 folded, f32r matmuls), gated top-k
fusion, residual, 1x1 head, bilinear 4x upsample. Host folds weights,
shards inputs, gathers outputs.
"""
import sys
sys.path.insert(0, '/opt/trn_rl_repo')
import numpy as np
from contextlib import ExitStack

import concourse.bass as bass
import concourse.bacc as bacc
import concourse.tile as tile
import concourse.mybir as mybir
from concourse.bass_utils import run_bass_kernel_spmd

F32 = mybir.dt.float32
F32R = mybir.dt.float32r
AF = mybir.ActivationFunctionType
ALU = mybir.AluOpType
AX = mybir.AxisListType

B, E, CH, H, W = 2, 5, 128, 96, 96
BN_EPS = 1e-5
PS = [0, 23, 47, 70]
RO, RI, WP = 26, 28, 98
FI = RI * WP          # 2744
FO = RO * WP          # 2548
CT = FI + 2           # 2746 (guard elems at both ends)
CTG = CT + 98         # hp tile with extra guard for strided row-pair views
CG, NG = 32, 4
NU = 41

# ---------------- host constants (input independent) ----------------
def _mask_sets():
    yy = np.arange(H, dtype=np.float64) - H // 2
    d = np.sqrt(yy[:, None] ** 2 + yy[None, :] ** 2)
    M = (d / d.max() >= 0.3).astype(np.float64)
    L0 = np.fft.ifftshift(1.0 - M)
    U = np.nonzero(L0.any(axis=1))[0]
    V = np.nonzero(L0.any(axis=0))[0]
    return U, V, L0[np.ix_(U, V)]

UIDX, VIDX, LSUB = _mask_sets()
assert len(UIDX) == NU and len(VIDX) == NU
UH = UIDX[UIDX <= 48]          # half u-spectrum (0..20): 21 rows
NUH = len(UH)
DBL = np.where(UH == 0, 1.0, 2.0)
_hh = np.arange(H, dtype=np.float64)
_cos = lambda a, b: np.cos(2 * np.pi * np.outer(a, b) / H)
_sin = lambda a, b: np.sin(2 * np.pi * np.outer(a, b) / H)

DFT_A = np.concatenate([_cos(UH, _hh).T, -_sin(UH, _hh).T], axis=1).astype(np.float32)          # [96, 42]
DFT_B = np.concatenate([_cos(VIDX, _hh).T, _sin(VIDX, _hh).T, -_sin(VIDX, _hh).T], axis=1).astype(np.float32)  # [96,123]
DFT_D = (np.concatenate([_cos(_hh, VIDX).T, -_sin(_hh, VIDX).T], axis=1) / (H * W)).astype(np.float32)  # [41,192]
LSUBH = LSUB[:NUH, :]                                    # [21u', 41v']
MASKREP = np.tile(LSUBH.T, (1, 16)).astype(np.float32)   # [41, 336]
IDENT96 = np.eye(96, dtype=np.float32)

def _up_mat(n, on):
    s = np.arange(on, dtype=np.float64) * ((n - 1) / (on - 1))
    i0 = np.floor(s).astype(int)
    i1 = np.minimum(i0 + 1, n - 1)
    t = s - i0
    Uy = np.zeros((on, n), np.float64)
    Uy[np.arange(on), i0] += 1 - t
    Uy[np.arange(on), i1] += t
    return Uy

UY_FULL = _up_mat(H, 4 * H)
UX_T = np.ascontiguousarray(_up_mat(W, 4 * W).T).astype(np.float32)  # [96, 384]

def _percore_dftc(ps):
    rows = np.arange(ps - 1, ps + RI - 1, dtype=np.float64)
    valid = (rows >= 0) & (rows < H)
    rr = np.where(valid, rows, 0.0)
    Cc = _cos(rr, UH) * valid[:, None] * DBL[None, :]
    Sc = _sin(rr, UH) * valid[:, None] * DBL[None, :]
    return np.concatenate([Cc.T, Sc.T, -Sc.T], axis=1).astype(np.float32)  # [21, 84]

def _percore_fmask(ps):
    m = np.zeros((RI, WP), np.float32)
    m[:, 1:97] = 1.0
    for i in range(RI):
        if not (0 <= ps - 1 + i < H):
            m[i, :] = 0.0
    return m.reshape(1, FI)

def _percore_uy(q):
    return np.ascontiguousarray(UY_FULL[96 * q:96 * (q + 1), PS[q]:PS[q] + RO].T).astype(np.float32)  # [26,96]


def _hp_rows2(hp, c0, y0):
    # [32, 2, 96] view of hp rows y0 and y0+2 (stride 2 rows), interior cols
    base = hp[c0:c0 + CG, 2 + y0 * WP: 2 + y0 * WP + 4 * WP]
    return base.rearrange("p (y w) -> p y w", y=2, w=2 * WP)[:, :, 0:96]

def _fp_rows2(fp, c0, y0):
    base = fp[c0:c0 + CG, 1 + y0 * WP: 1 + y0 * WP + 4 * WP]
    return base.rearrange("p (y w) -> p y w", y=2, w=2 * WP)[:, :, 0:96]

def _hp_rows(hp, c0, y0):
    # [32, 4, 96] view of hp rows y0..y0+3, interior cols (tile has +1 shift)
    base = hp[c0:c0 + CG, 2 + y0 * WP: 2 + y0 * WP + 4 * WP]
    return base.rearrange("p (y w) -> p y w", y=4)[:, :, 0:96]

def _fp_rows(fp, c0, y0):
    base = fp[c0:c0 + CG, 1 + y0 * WP: 1 + y0 * WP + 4 * WP]
    return base.rearrange("p (y w) -> p y w", y=4)[:, :, 0:96]

# ============================================================
def build_nc():
    nc = bacc.Bacc("TRN2", target_bir_lowering=False, debug=False, num_devices=8)

    def din(name, shape):
        return nc.dram_tensor(name, list(shape), F32, kind="ExternalInput").ap()

    I = {n: din(n, s) for n, s in [
        ('feats_pad', [CH, FI]), ('feats_fft', [H, CH * W]), ('fmask', [1, FI]),
        ('rts_r', [CH, 2 * 576]), ('rww', [CH, 20]), ('rbias', [1, 10]), ('uu', [1, 10]),
        ('dft_a', [H, 2 * NUH]), ('dft_b', [H, 123]), ('dft_c', [NUH, 84]), ('dft_d', [NU, 192]),
        ('maskrep', [NU, 16 * NUH]), ('ident', [96, 96]),
        ('wf', [4, CH, 9 * CH]), ('wz', [4, CH, 9 * CH]), ('pwz', [4, CH, CH]),
        ('bvecs', [CH, 25]), ('cow', [CH, 1]), ('uy', [RO, 96]), ('ux', [96, 384]),
    ]}
    o_up = nc.dram_tensor('o_up', [96, 384], F32, kind="ExternalOutput").ap()
    o_gf = nc.dram_tensor('o_gf', [1, E], F32, kind="ExternalOutput").ap()
    o_gz = nc.dram_tensor('o_gz', [1, E], F32, kind="ExternalOutput").ap()
    scratch = nc.dram_tensor('scratch', [RO, WP], F32).ap()

    with tile.TileContext(nc) as tc, ExitStack() as ctx:
        cpool = ctx.enter_context(tc.tile_pool(name="consts", bufs=1))
        big = ctx.enter_context(tc.tile_pool(name="big", bufs=1))
        gx = ctx.enter_context(tc.tile_pool(name="gx", bufs=2))
        fzp = ctx.enter_context(tc.tile_pool(name="fz", bufs=2))
        wpool = ctx.enter_context(tc.tile_pool(name="wts", bufs=2))
        fft = ctx.enter_context(tc.tile_pool(name="fft", bufs=1))
        small = ctx.enter_context(tc.tile_pool(name="small", bufs=2))
        ctp = ctx.enter_context(tc.tile_pool(name="ctp", bufs=2))
        pconv = ctx.enter_context(tc.tile_pool(name="pconv", bufs=3, space="PSUM"))
        pfft = ctx.enter_context(tc.tile_pool(name="pfft", bufs=3, space="PSUM"))
        pmisc = ctx.enter_context(tc.tile_pool(name="pmisc", bufs=2, space="PSUM"))

        def cload(name, shape, rdtype=None):
            t = cpool.tile(list(shape), F32, tag=name, name=name + '_t')
            nc.scalar.dma_start(t[:], I[name])
            if rdtype is None:
                return t
            tr = cpool.tile(list(shape), rdtype, tag=name + 'r', name=name + '_tr')
            nc.gpsimd.tensor_copy(tr[:], t[:])
            return tr

        dft_a = cload('dft_a', (H, 2 * NUH))                 # f32 (stage A is plain fp32)
        dft_b = cload('dft_b', (H, 123), F32R)
        dft_c = cload('dft_c', (NUH, 84), F32R)
        dft_d = cload('dft_d', (NU, 192), F32R)
        maskrep = cload('maskrep', (NU, 16 * NUH))
        ident_f = cpool.tile([96, 96], F32, tag='identf')
        nc.sync.dma_start(ident_f[:], I['ident'])
        ident = cpool.tile([96, 96], F32R, tag='identr')
        nc.gpsimd.tensor_copy(ident[:], ident_f[:])
        bvecs = cload('bvecs', (CH, 25))
        cow = cload('cow', (CH, 1))
        uy = cload('uy', (RO, 96))
        ux = cload('ux', (96, 384))

        xg_tiles = []
        for g in range(NG):
            xgt = fft.tile([H, CG * W], F32, tag='xg', bufs=2, name=f'xg{g}')
            hw = CG * W // 2
            nc.sync.dma_start(xgt[:, 0:hw], I['feats_fft'][:, g * CG * W:g * CG * W + hw])
            nc.sync.dma_start(xgt[:, hw:], I['feats_fft'][:, g * CG * W + hw:(g + 1) * CG * W])
            xg_tiles.append(xgt)
            if g == 0:
                feats_pad = big.tile([CH, FI + 100], F32)
                nc.sync.dma_start(feats_pad[:, 0:FI], I['feats_pad'])
        fmask_sm = cpool.tile([1, FI], F32, tag='fmask')
        nc.sync.dma_start(fmask_sm[:], I['fmask'])
        fmask_b = big.tile([CH, FI], F32)
        nc.gpsimd.partition_broadcast(fmask_b[:], fmask_sm[:])

        # ================= router =================
        rts_t = small.tile([CH, 2, 576], F32, tag='rts', bufs=1)
        nc.sync.dma_start(rts_t[:], I['rts_r'].rearrange("p (i s) -> p i s", i=2))
        rww_t = small.tile([CH, 20], F32, tag='rww', bufs=1)
        nc.sync.dma_start(rww_t[:], I['rww'])
        rbias_t = small.tile([1, 10], F32, tag='rbias', bufs=1)
        nc.sync.dma_start(rbias_t[:], I['rbias'])
        uu_t = small.tile([1, 10], F32, tag='uu', bufs=1)
        nc.sync.dma_start(uu_t[:], I['uu'])

        pooled = small.tile([CH, 2], F32, tag='pooled', bufs=1)
        nc.vector.tensor_reduce(pooled[:], rts_t[:], AX.X, ALU.add)
        nc.vector.tensor_scalar(pooled[:], pooled[:], 1.0 / 576.0, None, ALU.mult)

        for bi, br in enumerate(('f', 'z')):
            def st(tag, shape=(1, E)):
                return small.tile(list(shape), F32, tag=tag + br, bufs=1, name=tag + br)
            lg_ps = pmisc.tile([1, E], F32, tag='mps')
            for j in range(2):
                nc.tensor.matmul(lg_ps[:], pooled[:, j:j + 1],
                                 rww_t[:, 10 * bi + 5 * j:10 * bi + 5 * (j + 1)],
                                 start=(j == 0), stop=(j == 1))
            lg = st('lg')
            nc.vector.tensor_tensor(lg[:], lg_ps[:], rbias_t[:, 5 * bi:5 * bi + 5], ALU.add)
            mean = st('mean', (1, 1))
            nc.vector.tensor_reduce(mean[:], lg[:], AX.X, ALU.add)
            nc.vector.tensor_scalar(mean[:], mean[:], 1.0 / E, None, ALU.mult)
            xm = st('xm')
            nc.vector.tensor_scalar(xm[:], lg[:], mean[:], None, ALU.subtract)
            sq = st('sq')
            nc.scalar.square(sq[:], xm[:])
            var = st('var', (1, 1))
            nc.vector.tensor_reduce(var[:], sq[:], AX.X, ALU.add)
            nc.vector.tensor_scalar(var[:], var[:], 1.0 / (E - 1), None, ALU.mult)
            sd = st('sd', (1, 1))
            nc.scalar.sqrt(sd[:], var[:])
            nc.vector.tensor_scalar(sd[:], sd[:], 1e-6, None, ALU.add)
            rsd = st('rsd', (1, 1))
            nc.vector.reciprocal(rsd[:], sd[:])
            zn = st('zn')
            nc.vector.tensor_scalar(zn[:], xm[:], rsd[:], None, ALU.mult)
            z = st('zz')
            nc.vector.tensor_tensor(z[:], zn[:], uu_t[:, 5 * bi:5 * bi + 5], ALU.add)
            mx = st('mx', (1, 1))
            nc.vector.tensor_reduce(mx[:], z[:], AX.X, ALU.max)
            mxn = st('mxn', (1, 1))
            nc.scalar.mul(mxn[:], mx[:], -1.0)
            ez = st('ez')
            nc.scalar.activation(ez[:], z[:], AF.Exp, bias=mxn[:], scale=1.0)
            ssum = st('ssum', (1, 1))
            nc.vector.tensor_reduce(ssum[:], ez[:], AX.X, ALU.add)
            rs = st('rs', (1, 1))
            nc.vector.reciprocal(rs[:], ssum[:])
            gate = st('gate')
            nc.vector.tensor_scalar(gate[:], ez[:], rs[:], None, ALU.mult)
            nc.sync.dma_start(o_gf if br == 'f' else o_gz, gate[:])

        # ================= fuzzy shared =================
        lt = big.tile([CH, FI], F32)
        ta = gx.tile([CH, FI], F32, tag='gx', name='ta')
        tb = gx.tile([CH, FI], F32, tag='gx', name='tb')
        for h0, h1 in ((0, FI // 2), (FI // 2, FI)):
            s = slice(h0, h1)
            nc.vector.tensor_scalar(ta[:, s].bitcast(mybir.dt.int32),
                                    feats_pad[:, s].bitcast(mybir.dt.int32),
                                    0x7FFFFFFF, None, ALU.bitwise_and)
            nc.scalar.activation(tb[:, s], ta[:, s], AF.Exp, scale=-1.0)
            nc.vector.tensor_scalar(ta[:, s], tb[:, s], 2.0, None, ALU.add)
            nc.vector.reciprocal(lt[:, s], ta[:, s])
            nc.vector.tensor_tensor(ta[:, s], tb[:, s], lt[:, s], ALU.mult)
            nc.vector.tensor_tensor(tb[:, s], ta[:, s], ta[:, s], ALU.mult)
            nc.vector.tensor_scalar(lt[:, s], tb[:, s], 2.0 / 7.0, 2.0 / 5.0, ALU.mult, ALU.add)
            nc.gpsimd.tensor_tensor(lt[:, s], lt[:, s], tb[:, s], ALU.mult)
            nc.vector.tensor_scalar(lt[:, s], lt[:, s], 2.0 / 3.0, None, ALU.add)
            nc.gpsimd.tensor_tensor(lt[:, s], lt[:, s], tb[:, s], ALU.mult)
            nc.vector.tensor_scalar(lt[:, s], lt[:, s], 2.0, None, ALU.add)
            nc.vector.tensor_tensor(lt[:, s], lt[:, s], ta[:, s], ALU.mult)
            nc.vector.scalar_tensor_tensor(lt[:, s], feats_pad[:, s], 0.0, lt[:, s], ALU.max, ALU.add)
            nc.vector.tensor_scalar(lt[:, s], lt[:, s], 1.0000500025001668e-4, 9.210340371976182, ALU.max, ALU.min)

        acc = big.tile([CH, FO], F32)
        CS = [490, 490, 490, 490, 294, 294]
        CO = [0, 490, 980, 1470, 1960, 2254]

        def conv3x3(src_tile, wt_tile, scale_ap, bias_ap, first):
            for s in range(6):
                n = CS[s]
                ps = pconv.tile([CH, 512], F32, tag='cps')
                for t in range(9):
                    dy, dx = t // 3 - 1, t % 3 - 1
                    off = 1 + CO[s] + (1 + dy) * WP + dx
                    nc.tensor.matmul(ps[:, :n], wt_tile[:, t * CH:(t + 1) * CH],
                                     src_tile[:, off:off + n],
                                     start=(t == 0), stop=(t == 8))
                if first:
                    nc.scalar.activation(acc[:, CO[s]:CO[s] + n], ps[:, :n], AF.Relu,
                                         bias=bias_ap, scale=scale_ap)
                else:
                    tmp = ctp.tile([CH, 512], F32, tag='ctmp', bufs=2)
                    nc.scalar.activation(tmp[:, :n], ps[:, :n], AF.Relu,
                                         bias=bias_ap, scale=scale_ap)
                    eng = nc.gpsimd if (s % 2 == 0) else nc.vector
                    eng.tensor_tensor(acc[:, CO[s]:CO[s] + n], acc[:, CO[s]:CO[s] + n],
                                      tmp[:, :n], ALU.add)

        # ================= FFT -> hp =================
        hp = big.tile([CH, CTG], F32R)
        nc.vector.memset(hp[:].bitcast(mybir.dt.uint32), 0)

        def emit_fft_group(g):
            c0 = g * CG
            xg = xg_tiles[g]

            ztr = fft.tile([H, CG * NUH], F32R, tag='ztr', bufs=2)
            zti = fft.tile([H, CG * NUH], F32R, tag='zti', bufs=2)
            for cq in range(CG // 4):
                zps = pfft.tile([H, 4 * 2 * NUH], F32, tag='fps')
                for k in range(4):
                    ci = cq * 4 + k
                    nc.tensor.matmul(zps[:, 2 * NUH * k:2 * NUH * (k + 1)],
                                     xg[:, ci * W:(ci + 1) * W], dft_a[:],
                                     start=True, stop=True)
                zps3 = zps[:].rearrange("p (c r) -> p c r", c=4)
                nc.scalar.copy(
                    ztr[:, cq * 4 * NUH:(cq + 1) * 4 * NUH].rearrange("p (c u) -> p c u", c=4),
                    zps3[:, :, 0:NUH])
                nc.scalar.copy(
                    zti[:, cq * 4 * NUH:(cq + 1) * 4 * NUH].rearrange("p (c u) -> p c u", c=4),
                    zps3[:, :, NUH:2 * NUH])

            wm_r = fft.tile([NU, CG * NUH], F32, tag='fa1')
            wm_i = fft.tile([NU, CG * NUH], F32, tag='fa2')
            csz, cst = [16, 16], [0, 16]
            for ci, cn in zip(cst, csz):
                n = cn * NUH
                wr_ps = pfft.tile([NU, 492], F32, tag='fps')
                wi_ps = pfft.tile([NU, 492], F32, tag='fps')
                zr = ztr[:, ci * NUH:(ci + cn) * NUH]
                zi = zti[:, ci * NUH:(ci + cn) * NUH]
                nc.tensor.matmul(wr_ps[:, :n], dft_b[:, 0:41], zr, start=True, stop=False)
                nc.tensor.matmul(wr_ps[:, :n], dft_b[:, 41:82], zi, start=False, stop=True)
                nc.tensor.matmul(wi_ps[:, :n], dft_b[:, 0:41], zi, start=True, stop=False)
                nc.tensor.matmul(wi_ps[:, :n], dft_b[:, 82:123], zr, start=False, stop=True)
                nc.vector.tensor_tensor(wm_r[:, ci * NUH:ci * NUH + n], wr_ps[:, :n], maskrep[:, :n], ALU.mult)
                nc.vector.tensor_tensor(wm_i[:, ci * NUH:ci * NUH + n], wi_ps[:, :n], maskrep[:, :n], ALU.mult)

            wt_r = fft.tile([NUH, CG * NU], F32R, tag='fb1')
            wt_i = fft.tile([NUH, CG * NU], F32R, tag='fb2')
            tcsz, tcst = [12, 12, 8], [0, 12, 24]
            for src, dst in ((wm_r, wt_r), (wm_i, wt_i)):
                for ci, cn in zip(tcst, tcsz):
                    n = cn * NU
                    tp = pfft.tile([NUH, 492], F32, tag='fps')
                    for k in range(cn):
                        nc.tensor.transpose(tp[:, k * NU:(k + 1) * NU],
                                            src[:, (ci + k) * NUH:(ci + k + 1) * NUH],
                                            ident_f[0:NU, 0:NU])
                    nc.scalar.copy(dst[:, ci * NU:ci * NU + n], tp[:, :n])

            ar = fft.tile([RI, CG * NU], F32, tag='fa1')
            ai = fft.tile([RI, CG * NU], F32, tag='fa2')
            for ci, cn in zip(tcst, tcsz):
                n = cn * NU
                ar_ps = pfft.tile([RI, 492], F32, tag='fps')
                ai_ps = pfft.tile([RI, 492], F32, tag='fps')
                sl = slice(ci * NU, ci * NU + n)
                nc.tensor.matmul(ar_ps[:, :n], dft_c[:, 0:28], wt_r[:, sl], start=True, stop=False)
                nc.tensor.matmul(ar_ps[:, :n], dft_c[:, 56:84], wt_i[:, sl], start=False, stop=True)
                nc.tensor.matmul(ai_ps[:, :n], dft_c[:, 0:28], wt_i[:, sl], start=True, stop=False)
                nc.tensor.matmul(ai_ps[:, :n], dft_c[:, 28:56], wt_r[:, sl], start=False, stop=True)
                nc.scalar.copy(ar[:, sl], ar_ps[:, :n])
                nc.scalar.copy(ai[:, sl], ai_ps[:, :n])

            at_r = fft.tile([NU, CG * RI], F32R, tag='fb1')
            at_i = fft.tile([NU, CG * RI], F32R, tag='fb2')
            for src_t, dst in ((ar, at_r), (ai, at_i)):
                for half in range(2):
                    tp3 = pfft.tile([NU, 16 * RI], F32, tag='fps')
                    for k in range(16):
                        ci = half * 16 + k
                        nc.tensor.transpose(tp3[:, k * RI:(k + 1) * RI],
                                            src_t[:, ci * NU:(ci + 1) * NU],
                                            ident_f[0:RI, 0:RI])
                    nc.scalar.copy(
                        dst[:].rearrange("p (y c) -> p c y", c=CG)[:, half * 16:(half + 1) * 16, :],
                        tp3[:].rearrange("p (c y) -> p c y", c=16))

            lp_sb = fft.tile([96, CG * RI], F32R, tag='ztlp')
            for half in range(2):
                lp_ps = pfft.tile([96, 448], F32, tag='fps')
                sl = slice(half * 448, (half + 1) * 448)
                nc.tensor.matmul(lp_ps[:], dft_d[:, 0:96], at_r[:, sl], start=True, stop=False)
                nc.tensor.matmul(lp_ps[:], dft_d[:, 96:192], at_i[:, sl], start=False, stop=True)
                nc.scalar.copy(lp_sb[:, sl], lp_ps[:])

            for y4 in range(RI // 4):
                # two [96, 64] transposes cover 4 rows: psum partitions = (y-parity, c)
                t4 = pfft.tile([2 * CG, 2 * 96], F32R, tag='fps')
                y0 = y4 * 4
                nc.tensor.transpose(t4[:, 0:96],
                                    lp_sb[:, y0 * CG:(y0 + 2) * CG], ident[:, :])
                nc.tensor.transpose(t4[:, 96:192],
                                    lp_sb[:, (y0 + 2) * CG:(y0 + 4) * CG], ident[:, :])
                for par in range(2):
                    # psum rows [par*32:(par+1)*32] hold hp rows y0+par and y0+2+par
                    nc.vector.scalar_tensor_tensor(
                        _hp_rows2(hp, c0, y0 + par),
                        _fp_rows2(feats_pad, c0, y0 + par),
                        1.0,
                        t4[par * CG:(par + 1) * CG, :].rearrange("p (y w) -> p y w", y=2),
                        ALU.mult, ALU.subtract)

        def emit_fuzzy(e):
            pwz_t0 = wpool.tile([CH, CH], F32, tag='pwzt')
            nc.scalar.dma_start(pwz_t0[:], I['pwz'][e])
            pwz_t = wpool.tile([CH, CH], F32R, tag='pwztr')
            (nc.scalar.copy if e == 0 else nc.gpsimd.tensor_copy)(pwz_t[:], pwz_t0[:])
            xp = gx.tile([CH, FI], F32, tag='gx')
            XCS = [512, 512, 512, 512, 360, 346]
            XCO = [0, 512, 1024, 1536, 2048, 2398]
            for s in range(6):
                x0 = XCO[s]
                n = XCS[s]
                fpr = ctp.tile([CH, 512], F32R, tag='fpr', bufs=1)
                nc.gpsimd.tensor_copy(fpr[:, :n], feats_pad[:, x0:x0 + n])
                xps = pconv.tile([CH, 512], F32, tag='cps')
                nc.tensor.matmul(xps[:, :n], pwz_t[:], fpr[:, :n], start=True, stop=True)
                nc.vector.scalar_tensor_tensor(xp[:, x0:x0 + n], xps[:, :n],
                                               bvecs[:, 16 + e:17 + e],
                                               fmask_b[:, x0:x0 + n],
                                               ALU.add, ALU.mult)
            gt = gx.tile([CH, FI], F32, tag='gx')
            nc.scalar.activation(gt[:], lt[:], AF.Exp, scale=bvecs[:, 20 + e:21 + e])
            fz = fzp.tile([CH, CT], F32R, tag='fzt')
            nc.vector.memset(fz[:, 0:1].bitcast(mybir.dt.uint32), 0)
            nc.vector.memset(fz[:, CT - 1:CT].bitcast(mybir.dt.uint32), 0)
            nc.vector.tensor_tensor(fz[:, 1:1 + FI], gt[:], xp[:], ALU.mult)
            wz_t0 = wpool.tile([CH, 9 * CH], F32, tag='wct')
            nc.scalar.dma_start(wz_t0[:], I['wz'][e])
            wz_t = wpool.tile([CH, 9 * CH], F32R, tag='wctr')
            (nc.scalar.copy if e == 0 else nc.gpsimd.tensor_copy)(wz_t[:], wz_t0[:])
            conv3x3(fz, wz_t, bvecs[:, 8 + e:9 + e], bvecs[:, 12 + e:13 + e], first=(e == 0))


        for i in range(4):
            emit_fft_group(i)
            emit_fuzzy(i)

        for e in range(4):
            wf_t0 = wpool.tile([CH, 9 * CH], F32, tag='wct')
            nc.scalar.dma_start(wf_t0[:], I['wf'][e])
            wf_t = wpool.tile([CH, 9 * CH], F32R, tag='wctr')
            nc.gpsimd.tensor_copy(wf_t[:], wf_t0[:])
            conv3x3(hp, wf_t, bvecs[:, 0 + e:1 + e], bvecs[:, 4 + e:5 + e], first=False)

        # ================= head =================
        nc.vector.tensor_tensor(acc[:], acc[:], feats_pad[:, WP:WP + FO], ALU.add)

        o1 = gx.tile([1, FO], F32, tag='gx', name='o1')
        for s in range(6):
            n = CS[s]
            cps = pmisc.tile([1, 512], F32, tag='mps')
            nc.tensor.matmul(cps[:, :n], cow[:], acc[:, CO[s]:CO[s] + n], start=True, stop=True)
            nc.scalar.activation(o1[:, CO[s]:CO[s] + n], cps[:, :n], AF.Identity,
                                 bias=bvecs[0:1, 24:25], scale=1.0)
            nc.sync.dma_start(scratch.rearrange("a b -> (a b)")[CO[s]:CO[s] + n],
                              o1[:, CO[s]:CO[s] + n])

        r_in = small.tile([RO, WP], F32, tag='rin', bufs=1)
        nc.sync.dma_start(r_in[0:13, :], scratch[0:13, :])
        nc.sync.dma_start(r_in[13:26, :], scratch[13:26, :])

        up1 = pmisc.tile([96, 96], F32, tag='mps')
        nc.tensor.matmul(up1[:], r_in[:, 1:97], uy[:], start=True, stop=True)
        rt2 = small.tile([96, 96], F32, tag='rt2', bufs=1)
        nc.vector.tensor_copy(rt2[:], up1[:])
        up2 = pmisc.tile([96, 384], F32, tag='mps')
        nc.tensor.matmul(up2[:], rt2[:], ux[:], start=True, stop=True)
        oup = small.tile([96, 384], F32, tag='oup', bufs=1)
        nc.vector.tensor_copy(oup[:], up2[:])
        nc.sync.dma_start(o_up, oup[:])

    nc.compile()
    return nc


# ============================================================
_NC_CACHE = {}

def _get_nc():
    if 'nc' not in _NC_CACHE:
        _NC_CACHE['nc'] = build_nc()
    return _NC_CACHE['nc']


def _host_router(rts_b, w, b, u):
    pooled = rts_b.reshape(256, -1).astype(np.float64).mean(axis=1)
    logits = w.astype(np.float64) @ pooled + b
    mu = logits.mean()
    sd = logits.std(ddof=1)
    z = (logits - mu) / (sd + 1e-6) - np.log(-np.log(u + 1e-9) + 1e-9)
    z = z - z.max()
    p = np.exp(z)
    return p / p.sum()


def _fold_host(inputs):
    f = {}
    s_f = inputs['fg_g'] / np.sqrt(inputs['fg_v'] + BN_EPS)
    Wf = np.einsum('eoikl,eic->eockl', inputs['fg_cw'], inputs['fg_pw'])
    Wf = Wf * s_f[:, :, None, None, None]
    bf = (inputs['fg_cb'] - inputs['fg_m']) * s_f + inputs['fg_beta']
    f['wf'] = np.ascontiguousarray(Wf.transpose(0, 2, 3, 4, 1).reshape(E, CH, 9 * CH)).astype(np.float32)
    s_z = inputs['iz_g'] / np.sqrt(inputs['iz_v'] + BN_EPS)
    Wz = inputs['iz_cw'] * s_z[:, :, None, None, None]
    f['wz'] = np.ascontiguousarray(Wz.transpose(0, 2, 3, 4, 1).reshape(E, CH, 9 * CH)).astype(np.float32)
    bz = (inputs['iz_cb'] - inputs['iz_m']) * s_z + inputs['iz_beta']
    a = inputs['iz_a'][:, None]
    f['pwz'] = np.ascontiguousarray((inputs['iz_pw'] * a[:, :, None]).transpose(0, 2, 1)).astype(np.float32)
    pbz = inputs['iz_pb'] * a
    rf = inputs['rf_w'].reshape(E, 2, CH).transpose(2, 1, 0)
    rz = inputs['rz_w'].reshape(E, 2, CH).transpose(2, 1, 0)
    f['rww'] = np.concatenate([rf.reshape(CH, 10), rz.reshape(CH, 10)], axis=1).astype(np.float32)
    f['rbias'] = np.concatenate([inputs['rf_b'], inputs['rz_b']])[None].astype(np.float32)
    f['bf'] = bf
    f['bz'] = bz
    f['pbz'] = pbz
    f['mq'] = -np.exp(inputs['iz_p_raw'])
    f['cob'] = inputs['co_b'][0]
    f['cow'] = np.ascontiguousarray(inputs['co_w'].T).astype(np.float32)
    # per-b routing (host mirror; device still computes gate outputs itself)
    f['sel'] = {}
    for b in range(B):
        pf = _host_router(inputs['rts'][b], inputs['rf_w'], inputs['rf_b'], inputs['u_fourier'][b])
        pz = _host_router(inputs['rts'][b], inputs['rz_w'], inputs['rz_b'], inputs['u_fuzzy'][b])
        sf = [e for e in range(E) if e != int(np.argmin(pf))]
        sz = [e for e in range(E) if e != int(np.argmin(pz))]
        f['sel'][b] = (sf, pf, sz, pz)
    return f


def _core_inputs(inputs, f, core):
    b, q = core // 4, core % 4
    ps = PS[q]
    sf, pf, sz, pz = f['sel'][b]
    bv = np.zeros((CH, 25), np.float32)
    bv[:, 0:4] = np.float32(pf[sf])[None, :]
    bv[:, 4:8] = (pf[sf][None, :] * f['bf'][sf].T).astype(np.float32)
    bv[:, 8:12] = np.float32(pz[sz])[None, :]
    bv[:, 12:16] = (pz[sz][None, :] * f['bz'][sz].T).astype(np.float32)
    bv[:, 16:20] = f['pbz'][sz].T
    bv[:, 20:24] = f['mq'][None, sz]
    bv[:, 24] = f['cob']
    feats = inputs['feats'][b]
    fp = np.zeros((CH, RI, WP), np.float32)
    r0, r1 = ps - 1, ps + RI - 1
    sr0, sr1 = max(r0, 0), min(r1, H)
    fp[:, sr0 - r0:sr1 - r0, 1:97] = feats[:, sr0:sr1, :]
    d = {
        'feats_pad': fp.reshape(CH, FI),
        'feats_fft': feats.transpose(1, 0, 2).reshape(H, CH * W),
        'fmask': _percore_fmask(ps),
        'rts_r': inputs['rts'][b].reshape(2, CH, 576).transpose(1, 0, 2).reshape(CH, 2 * 576),
        'rww': f['rww'], 'rbias': f['rbias'],
        'uu': -np.log(-np.log(np.concatenate([inputs['u_fourier'][b], inputs['u_fuzzy'][b]])[None] + 1e-9) + 1e-9),
        'dft_a': DFT_A, 'dft_b': DFT_B, 'dft_c': _percore_dftc(ps), 'dft_d': DFT_D,
        'maskrep': MASKREP, 'ident': IDENT96,
        'wf': f['wf'][sf], 'wz': f['wz'][sz], 'pwz': f['pwz'][sz],
        'bvecs': bv, 'cow': f['cow'],
        'uy': _percore_uy(q), 'ux': UX_T,
    }
    return {k: np.ascontiguousarray(v, dtype=np.float32) for k, v in d.items()}


def make_in_maps(inputs):
    inputs = {k: np.asarray(v, dtype=np.float32) for k, v in inputs.items()}
    f = _fold_host(inputs)
    return [_core_inputs(inputs, f, c) for c in range(8)]


def assemble(results):
    out = np.zeros((B, 1, 4 * H, 4 * W), np.float32)
    gf = np.zeros((B, E), np.float32)
    gz = np.zeros((B, E), np.float32)
    for c in range(8):
        b, q = c // 4, c % 4
        out[b, 0, 96 * q:96 * (q + 1), :] = results[c]['o_up']
        if q == 0:
            gf[b] = results[c]['o_gf'][0]
            gz[b] = results[c]['o_gz'][0]
    return out, gf, gz


class _Exec:
    """Cached 8-core PJRT executor (jit built once, reused across calls)."""

    def __init__(self, nc):
        import jax
        from jax.sharding import Mesh, PartitionSpec
        from jax.experimental.shard_map import shard_map
        from concourse import mybir as _mb
        from concourse.bass2jax import (_bass_exec_p, partition_id_tensor,
                                        install_neuronx_cc_hook)
        install_neuronx_cc_hook()
        self.jax = jax
        in_names, out_names, out_avals, zero_outs = [], [], [], []
        pname = nc.partition_id_tensor.name if nc.partition_id_tensor else None
        for alloc in nc.m.functions[0].allocations:
            if not isinstance(alloc, _mb.MemoryLocationSet):
                continue
            name = alloc.memorylocations[0].name
            if alloc.kind == "ExternalInput":
                if name != pname:
                    in_names.append(name)
            elif alloc.kind == "ExternalOutput":
                shape = list(alloc.tensor_shape)
                np_dt = _mb.dt.np(alloc.dtype)
                out_avals.append(jax.core.ShapedArray(shape, np_dt))
                zero_outs.append(np.zeros(shape, np_dt))
                out_names.append(name)
        self.in_names, self.out_names = list(in_names), out_names
        self.zero_outs = zero_outs
        n_params, n_outs = len(in_names), len(out_names)
        all_names = in_names + out_names + ([pname] if pname else [])

        def _body(*args):
            operands = list(args)
            if pname is not None:
                operands.append(partition_id_tensor())
            outs = _bass_exec_p.bind(
                *operands, out_avals=tuple(out_avals), in_names=tuple(all_names),
                out_names=tuple(out_names), lowering_input_output_aliases=(),
                sim_require_finite=True, sim_require_nnan=True, nc=nc)
            return tuple(outs)

        devices = jax.devices()[:8]
        mesh = Mesh(np.asarray(devices), ("core",))
        in_specs = (PartitionSpec("core"),) * (n_params + n_outs)
        out_specs = (PartitionSpec("core"),) * n_outs
        self.sharded = jax.jit(
            shard_map(_body, mesh=mesh, in_specs=in_specs, out_specs=out_specs,
                      check_rep=False),
            donate_argnums=tuple(range(n_params, n_params + n_outs)),
            keep_unused=True)

    def concat_inputs(self, in_maps):
        return [np.concatenate([np.asarray(m[n]) for m in in_maps], axis=0)
                for n in self.in_names]

    def zeros(self):
        return [np.zeros((8 * z.shape[0], *z.shape[1:]), z.dtype) for z in self.zero_outs]

    def run(self, concat_in):
        outs = self.sharded(*concat_in, *self.zeros())
        return outs

    def to_results(self, outs):
        res = []
        for c in range(8):
            d = {}
            for i, n in enumerate(self.out_names):
                a = np.asarray(outs[i])
                per = a.shape[0] // 8
                d[n] = a[c * per:(c + 1) * per]
            res.append(d)
        return res


def _get_exec():
    if 'exec' not in _NC_CACHE:
        _NC_CACHE['exec'] = _Exec(_get_nc())
    return _NC_CACHE['exec']


def kernel(**inputs):
    ex = _get_exec()
    in_maps = make_in_maps(inputs)
    outs = ex.run(ex.concat_inputs(in_maps))
    return assemble(ex.to_results(outs))


# revision 44
# speedup vs baseline: 1.0357x; 1.0357x over previous
"""Trainium2 Bass kernel for nn_BACMoE_Model (moe_routing).

Self-contained. Strategy (8 NeuronCores, SPMD): core i = (b = i//4,
row-quarter q = i%4). Each core computes its 26-row output window (+halo):
routers (replicated), high-pass FFT filter via 41-frequency-restricted DFT
matmuls, 10 expert branches (BN/1x1 folded, f32r matmuls), gated top-k
fusion, residual, 1x1 head, bilinear 4x upsample. Host folds weights,
shards inputs, gathers outputs.
"""
import sys
sys.path.insert(0, '/opt/trn_rl_repo')
import numpy as np
from contextlib import ExitStack

import concourse.bass as bass
import concourse.bacc as bacc
import concourse.tile as tile
import concourse.mybir as mybir
from concourse.bass_utils import run_bass_kernel_spmd

F32 = mybir.dt.float32
F32R = mybir.dt.float32r
AF = mybir.ActivationFunctionType
ALU = mybir.AluOpType
AX = mybir.AxisListType

B, E, CH, H, W = 2, 5, 128, 96, 96
BN_EPS = 1e-5
PS = [0, 23, 47, 70]
RO, RI, WP = 26, 28, 98
FI = RI * WP          # 2744
FO = RO * WP          # 2548
CT = FI + 2           # 2746 (guard elems at both ends)
CTG = CT + 98         # hp tile with extra guard for strided row-pair views
CG, NG = 32, 4
NU = 41

# ---------------- host constants (input independent) ----------------
def _mask_sets():
    yy = np.arange(H, dtype=np.float64) - H // 2
    d = np.sqrt(yy[:, None] ** 2 + yy[None, :] ** 2)
    M = (d / d.max() >= 0.3).astype(np.float64)
    L0 = np.fft.ifftshift(1.0 - M)
    U = np.nonzero(L0.any(axis=1))[0]
    V = np.nonzero(L0.any(axis=0))[0]
    return U, V, L0[np.ix_(U, V)]

UIDX, VIDX, LSUB = _mask_sets()
assert len(UIDX) == NU and len(VIDX) == NU
UH = UIDX[UIDX <= 48]          # half u-spectrum (0..20): 21 rows
NUH = len(UH)
DBL = np.where(UH == 0, 1.0, 2.0)
_hh = np.arange(H, dtype=np.float64)
_cos = lambda a, b: np.cos(2 * np.pi * np.outer(a, b) / H)
_sin = lambda a, b: np.sin(2 * np.pi * np.outer(a, b) / H)

DFT_A = np.concatenate([_cos(UH, _hh).T, -_sin(UH, _hh).T], axis=1).astype(np.float32)          # [96, 42]
DFT_B = np.concatenate([_cos(VIDX, _hh).T, _sin(VIDX, _hh).T, -_sin(VIDX, _hh).T], axis=1).astype(np.float32)  # [96,123]
DFT_D = (np.concatenate([_cos(_hh, VIDX).T, -_sin(_hh, VIDX).T], axis=1) / (H * W)).astype(np.float32)  # [41,192]
LSUBH = LSUB[:NUH, :]                                    # [21u', 41v']
MASKREP = np.tile(LSUBH.T, (1, 16)).astype(np.float32)   # [41, 336]
IDENT96 = np.eye(96, dtype=np.float32)

def _up_mat(n, on):
    s = np.arange(on, dtype=np.float64) * ((n - 1) / (on - 1))
    i0 = np.floor(s).astype(int)
    i1 = np.minimum(i0 + 1, n - 1)
    t = s - i0
    Uy = np.zeros((on, n), np.float64)
    Uy[np.arange(on), i0] += 1 - t
    Uy[np.arange(on), i1] += t
    return Uy

UY_FULL = _up_mat(H, 4 * H)
UX_T = np.ascontiguousarray(_up_mat(W, 4 * W).T).astype(np.float32)  # [96, 384]

def _percore_dftc(ps):
    rows = np.arange(ps - 1, ps + RI - 1, dtype=np.float64)
    valid = (rows >= 0) & (rows < H)
    rr = np.where(valid, rows, 0.0)
    Cc = _cos(rr, UH) * valid[:, None] * DBL[None, :]
    Sc = _sin(rr, UH) * valid[:, None] * DBL[None, :]
    return np.concatenate([Cc.T, Sc.T, -Sc.T], axis=1).astype(np.float32)  # [21, 84]

def _percore_fmask(ps):
    m = np.zeros((RI, WP), np.float32)
    m[:, 1:97] = 1.0
    for i in range(RI):
        if not (0 <= ps - 1 + i < H):
            m[i, :] = 0.0
    return m.reshape(1, FI)

def _percore_uy(q):
    return np.ascontiguousarray(UY_FULL[96 * q:96 * (q + 1), PS[q]:PS[q] + RO].T).astype(np.float32)  # [26,96]


def _hp_rows2(hp, c0, y0):
    # [32, 2, 96] view of hp rows y0 and y0+2 (stride 2 rows), interior cols
    base = hp[c0:c0 + CG, 2 + y0 * WP: 2 + y0 * WP + 4 * WP]
    return base.rearrange("p (y w) -> p y w", y=2, w=2 * WP)[:, :, 0:96]

def _fp_rows2(fp, c0, y0):
    base = fp[c0:c0 + CG, 1 + y0 * WP: 1 + y0 * WP + 4 * WP]
    return base.rearrange("p (y w) -> p y w", y=2, w=2 * WP)[:, :, 0:96]

def _hp_rows(hp, c0, y0):
    # [32, 4, 96] view of hp rows y0..y0+3, interior cols (tile has +1 shift)
    base = hp[c0:c0 + CG, 2 + y0 * WP: 2 + y0 * WP + 4 * WP]
    return base.rearrange("p (y w) -> p y w", y=4)[:, :, 0:96]

def _fp_rows(fp, c0, y0):
    base = fp[c0:c0 + CG, 1 + y0 * WP: 1 + y0 * WP + 4 * WP]
    return base.rearrange("p (y w) -> p y w", y=4)[:, :, 0:96]

# ============================================================
def build_nc():
    nc = bacc.Bacc("TRN2", target_bir_lowering=False, debug=False, num_devices=8)

    def din(name, shape):
        return nc.dram_tensor(name, list(shape), F32, kind="ExternalInput").ap()

    I = {n: din(n, s) for n, s in [
        ('feats_pad', [CH, FI]), ('feats_fft', [H, CH * W]), ('fmask', [1, FI]),
        ('rts_r', [CH, 2 * 576]), ('rww', [CH, 20]), ('rbias', [1, 10]), ('uu', [1, 10]),
        ('dft_a', [H, 2 * NUH]), ('dft_b', [H, 123]), ('dft_c', [NUH, 84]), ('dft_d', [NU, 192]),
        ('maskrep', [NU, 16 * NUH]), ('ident', [96, 96]),
        ('wf', [4, CH, 9 * CH]), ('wz', [4, CH, 9 * CH]), ('pwz', [4, CH, CH]),
        ('bvecs', [CH, 25]), ('cow', [CH, 1]), ('uy', [RO, 96]), ('ux', [96, 384]),
    ]}
    o_up = nc.dram_tensor('o_up', [96, 384], F32, kind="ExternalOutput").ap()
    o_gf = nc.dram_tensor('o_gf', [1, E], F32, kind="ExternalOutput").ap()
    o_gz = nc.dram_tensor('o_gz', [1, E], F32, kind="ExternalOutput").ap()
    scratch = nc.dram_tensor('scratch', [RO, WP], F32).ap()

    with tile.TileContext(nc) as tc, ExitStack() as ctx:
        cpool = ctx.enter_context(tc.tile_pool(name="consts", bufs=1))
        big = ctx.enter_context(tc.tile_pool(name="big", bufs=1))
        gx = ctx.enter_context(tc.tile_pool(name="gx", bufs=2))
        fzp = ctx.enter_context(tc.tile_pool(name="fz", bufs=2))
        wpool = ctx.enter_context(tc.tile_pool(name="wts", bufs=2))
        fft = ctx.enter_context(tc.tile_pool(name="fft", bufs=1))
        small = ctx.enter_context(tc.tile_pool(name="small", bufs=2))
        ctp = ctx.enter_context(tc.tile_pool(name="ctp", bufs=2))
        pconv = ctx.enter_context(tc.tile_pool(name="pconv", bufs=3, space="PSUM"))
        pfft = ctx.enter_context(tc.tile_pool(name="pfft", bufs=3, space="PSUM"))
        pmisc = ctx.enter_context(tc.tile_pool(name="pmisc", bufs=2, space="PSUM"))

        def cload(name, shape, rdtype=None):
            t = cpool.tile(list(shape), F32, tag=name, name=name + '_t')
            nc.scalar.dma_start(t[:], I[name])
            if rdtype is None:
                return t
            tr = cpool.tile(list(shape), rdtype, tag=name + 'r', name=name + '_tr')
            nc.gpsimd.tensor_copy(tr[:], t[:])
            return tr

        dft_a = cload('dft_a', (H, 2 * NUH))                 # f32 (stage A is plain fp32)
        dft_b = cload('dft_b', (H, 123), F32R)
        dft_c = cload('dft_c', (NUH, 84), F32R)
        dft_d = cload('dft_d', (NU, 192), F32R)
        maskrep = cload('maskrep', (NU, 16 * NUH))
        ident_f = cpool.tile([96, 96], F32, tag='identf')
        nc.sync.dma_start(ident_f[:], I['ident'])
        ident = cpool.tile([96, 96], F32R, tag='identr')
        nc.gpsimd.tensor_copy(ident[:], ident_f[:])
        bvecs = cload('bvecs', (CH, 25))
        cow = cload('cow', (CH, 1))
        uy = cload('uy', (RO, 96))
        ux = cload('ux', (96, 384))

        xg_tiles = []
        for g in range(NG):
            xgt = fft.tile([H, CG * W], F32, tag='xg', bufs=2, name=f'xg{g}')
            hw = CG * W // 2
            nc.sync.dma_start(xgt[:, 0:hw], I['feats_fft'][:, g * CG * W:g * CG * W + hw])
            nc.sync.dma_start(xgt[:, hw:], I['feats_fft'][:, g * CG * W + hw:(g + 1) * CG * W])
            xg_tiles.append(xgt)
            if g == 0:
                feats_pad = big.tile([CH, FI + 100], F32)
                nc.sync.dma_start(feats_pad[:, 0:FI], I['feats_pad'])
        fmask_sm = cpool.tile([1, FI], F32, tag='fmask')
        nc.sync.dma_start(fmask_sm[:], I['fmask'])
        fmask_b = big.tile([CH, FI], F32)
        nc.gpsimd.partition_broadcast(fmask_b[:], fmask_sm[:])

        # ================= router =================
        rts_t = small.tile([CH, 2, 576], F32, tag='rts', bufs=1)
        nc.sync.dma_start(rts_t[:], I['rts_r'].rearrange("p (i s) -> p i s", i=2))
        rww_t = small.tile([CH, 20], F32, tag='rww', bufs=1)
        nc.sync.dma_start(rww_t[:], I['rww'])
        rbias_t = small.tile([1, 10], F32, tag='rbias', bufs=1)
        nc.sync.dma_start(rbias_t[:], I['rbias'])
        uu_t = small.tile([1, 10], F32, tag='uu', bufs=1)
        nc.sync.dma_start(uu_t[:], I['uu'])

        pooled = small.tile([CH, 2], F32, tag='pooled', bufs=1)
        nc.vector.tensor_reduce(pooled[:], rts_t[:], AX.X, ALU.add)
        nc.vector.tensor_scalar(pooled[:], pooled[:], 1.0 / 576.0, None, ALU.mult)

        for bi, br in enumerate(('f', 'z')):
            def st(tag, shape=(1, E)):
                return small.tile(list(shape), F32, tag=tag + br, bufs=1, name=tag + br)
            lg_ps = pmisc.tile([1, E], F32, tag='mps')
            for j in range(2):
                nc.tensor.matmul(lg_ps[:], pooled[:, j:j + 1],
                                 rww_t[:, 10 * bi + 5 * j:10 * bi + 5 * (j + 1)],
                                 start=(j == 0), stop=(j == 1))
            lg = st('lg')
            nc.vector.tensor_tensor(lg[:], lg_ps[:], rbias_t[:, 5 * bi:5 * bi + 5], ALU.add)
            mean = st('mean', (1, 1))
            nc.vector.tensor_reduce(mean[:], lg[:], AX.X, ALU.add)
            nc.vector.tensor_scalar(mean[:], mean[:], 1.0 / E, None, ALU.mult)
            xm = st('xm')
            nc.vector.tensor_scalar(xm[:], lg[:], mean[:], None, ALU.subtract)
            sq = st('sq')
            nc.scalar.square(sq[:], xm[:])
            var = st('var', (1, 1))
            nc.vector.tensor_reduce(var[:], sq[:], AX.X, ALU.add)
            nc.vector.tensor_scalar(var[:], var[:], 1.0 / (E - 1), None, ALU.mult)
            sd = st('sd', (1, 1))
            nc.scalar.sqrt(sd[:], var[:])
            nc.vector.tensor_scalar(sd[:], sd[:], 1e-6, None, ALU.add)
            rsd = st('rsd', (1, 1))
            nc.vector.reciprocal(rsd[:], sd[:])
            zn = st('zn')
            nc.vector.tensor_scalar(zn[:], xm[:], rsd[:], None, ALU.mult)
            z = st('zz')
            nc.vector.tensor_tensor(z[:], zn[:], uu_t[:, 5 * bi:5 * bi + 5], ALU.add)
            mx = st('mx', (1, 1))
            nc.vector.tensor_reduce(mx[:], z[:], AX.X, ALU.max)
            mxn = st('mxn', (1, 1))
            nc.scalar.mul(mxn[:], mx[:], -1.0)
            ez = st('ez')
            nc.scalar.activation(ez[:], z[:], AF.Exp, bias=mxn[:], scale=1.0)
            ssum = st('ssum', (1, 1))
            nc.vector.tensor_reduce(ssum[:], ez[:], AX.X, ALU.add)
            rs = st('rs', (1, 1))
            nc.vector.reciprocal(rs[:], ssum[:])
            gate = st('gate')
            nc.vector.tensor_scalar(gate[:], ez[:], rs[:], None, ALU.mult)
            nc.sync.dma_start(o_gf if br == 'f' else o_gz, gate[:])

        # ================= fuzzy shared =================
        lt = big.tile([CH, FI], F32)
        ta = gx.tile([CH, FI], F32, tag='gx', name='ta')
        tb = gx.tile([CH, FI], F32, tag='gx', name='tb')
        for h0, h1 in ((0, FI // 2), (FI // 2, FI)):
            s = slice(h0, h1)
            nc.vector.tensor_scalar(ta[:, s].bitcast(mybir.dt.int32),
                                    feats_pad[:, s].bitcast(mybir.dt.int32),
                                    0x7FFFFFFF, None, ALU.bitwise_and)
            nc.scalar.activation(tb[:, s], ta[:, s], AF.Exp, scale=-1.0)
            nc.vector.tensor_scalar(ta[:, s], tb[:, s], 2.0, None, ALU.add)
            nc.vector.reciprocal(lt[:, s], ta[:, s])
            nc.vector.tensor_tensor(ta[:, s], tb[:, s], lt[:, s], ALU.mult)
            nc.vector.tensor_tensor(tb[:, s], ta[:, s], ta[:, s], ALU.mult)
            nc.vector.tensor_scalar(lt[:, s], tb[:, s], 2.0 / 7.0, 2.0 / 5.0, ALU.mult, ALU.add)
            nc.gpsimd.tensor_tensor(lt[:, s], lt[:, s], tb[:, s], ALU.mult)
            nc.vector.tensor_scalar(lt[:, s], lt[:, s], 2.0 / 3.0, None, ALU.add)
            nc.gpsimd.tensor_tensor(lt[:, s], lt[:, s], tb[:, s], ALU.mult)
            nc.vector.tensor_scalar(lt[:, s], lt[:, s], 2.0, None, ALU.add)
            nc.vector.tensor_tensor(lt[:, s], lt[:, s], ta[:, s], ALU.mult)
            nc.vector.scalar_tensor_tensor(lt[:, s], feats_pad[:, s], 0.0, lt[:, s], ALU.max, ALU.add)
            nc.vector.tensor_scalar(lt[:, s], lt[:, s], 1.0000500025001668e-4, 9.210340371976182, ALU.max, ALU.min)

        acc = big.tile([CH, FO], F32)
        CS = [490, 490, 490, 490, 294, 294]
        CO = [0, 490, 980, 1470, 1960, 2254]

        def conv3x3(src_tile, wt_tile, scale_ap, bias_ap, first):
            for s in range(6):
                n = CS[s]
                ps = pconv.tile([CH, 512], F32, tag='cps')
                for t in range(9):
                    dy, dx = t // 3 - 1, t % 3 - 1
                    off = 1 + CO[s] + (1 + dy) * WP + dx
                    nc.tensor.matmul(ps[:, :n], wt_tile[:, t * CH:(t + 1) * CH],
                                     src_tile[:, off:off + n],
                                     start=(t == 0), stop=(t == 8))
                if first:
                    nc.scalar.activation(acc[:, CO[s]:CO[s] + n], ps[:, :n], AF.Relu,
                                         bias=bias_ap, scale=scale_ap)
                else:
                    tmp = ctp.tile([CH, 512], F32, tag='ctmp', bufs=2)
                    nc.scalar.activation(tmp[:, :n], ps[:, :n], AF.Relu,
                                         bias=bias_ap, scale=scale_ap)
                    eng = nc.gpsimd if (s % 2 == 0) else nc.vector
                    eng.tensor_tensor(acc[:, CO[s]:CO[s] + n], acc[:, CO[s]:CO[s] + n],
                                      tmp[:, :n], ALU.add)

        # ================= FFT -> hp =================
        hp = big.tile([CH, CTG], F32R)
        nc.vector.memset(hp[:].bitcast(mybir.dt.uint32), 0)

        def emit_fft_group(g):
            c0 = g * CG
            xg = xg_tiles[g]

            ztr = fft.tile([H, CG * NUH], F32R, tag='ztr', bufs=2)
            zti = fft.tile([H, CG * NUH], F32R, tag='zti', bufs=2)
            for cq in range(CG // 4):
                zps = pfft.tile([H, 4 * 2 * NUH], F32, tag='fps')
                for k in range(4):
                    ci = cq * 4 + k
                    nc.tensor.matmul(zps[:, 2 * NUH * k:2 * NUH * (k + 1)],
                                     xg[:, ci * W:(ci + 1) * W], dft_a[:],
                                     start=True, stop=True)
                zps3 = zps[:].rearrange("p (c r) -> p c r", c=4)
                nc.scalar.copy(
                    ztr[:, cq * 4 * NUH:(cq + 1) * 4 * NUH].rearrange("p (c u) -> p c u", c=4),
                    zps3[:, :, 0:NUH])
                nc.scalar.copy(
                    zti[:, cq * 4 * NUH:(cq + 1) * 4 * NUH].rearrange("p (c u) -> p c u", c=4),
                    zps3[:, :, NUH:2 * NUH])

            wm_r = fft.tile([NU, CG * NUH], F32, tag='fa1')
            wm_i = fft.tile([NU, CG * NUH], F32, tag='fa2')
            csz, cst = [16, 16], [0, 16]
            for ci, cn in zip(cst, csz):
                n = cn * NUH
                wr_ps = pfft.tile([NU, 492], F32, tag='fps')
                wi_ps = pfft.tile([NU, 492], F32, tag='fps')
                zr = ztr[:, ci * NUH:(ci + cn) * NUH]
                zi = zti[:, ci * NUH:(ci + cn) * NUH]
                nc.tensor.matmul(wr_ps[:, :n], dft_b[:, 0:41], zr, start=True, stop=False)
                nc.tensor.matmul(wr_ps[:, :n], dft_b[:, 41:82], zi, start=False, stop=True)
                nc.tensor.matmul(wi_ps[:, :n], dft_b[:, 0:41], zi, start=True, stop=False)
                nc.tensor.matmul(wi_ps[:, :n], dft_b[:, 82:123], zr, start=False, stop=True)
                nc.vector.tensor_tensor(wm_r[:, ci * NUH:ci * NUH + n], wr_ps[:, :n], maskrep[:, :n], ALU.mult)
                nc.vector.tensor_tensor(wm_i[:, ci * NUH:ci * NUH + n], wi_ps[:, :n], maskrep[:, :n], ALU.mult)

            wt_r = fft.tile([NUH, CG * NU], F32R, tag='fb1')
            wt_i = fft.tile([NUH, CG * NU], F32R, tag='fb2')
            tcsz, tcst = [12, 12, 8], [0, 12, 24]
            for src, dst in ((wm_r, wt_r), (wm_i, wt_i)):
                for ci, cn in zip(tcst, tcsz):
                    n = cn * NU
                    tp = pfft.tile([NUH, 492], F32, tag='fps')
                    for k in range(cn):
                        nc.tensor.transpose(tp[:, k * NU:(k + 1) * NU],
                                            src[:, (ci + k) * NUH:(ci + k + 1) * NUH],
                                            ident_f[0:NU, 0:NU])
                    nc.scalar.copy(dst[:, ci * NU:ci * NU + n], tp[:, :n])

            ar = fft.tile([RI, CG * NU], F32, tag='fa1')
            ai = fft.tile([RI, CG * NU], F32, tag='fa2')
            for ci, cn in zip(tcst, tcsz):
                n = cn * NU
                ar_ps = pfft.tile([RI, 492], F32, tag='fps')
                ai_ps = pfft.tile([RI, 492], F32, tag='fps')
                sl = slice(ci * NU, ci * NU + n)
                nc.tensor.matmul(ar_ps[:, :n], dft_c[:, 0:28], wt_r[:, sl], start=True, stop=False)
                nc.tensor.matmul(ar_ps[:, :n], dft_c[:, 56:84], wt_i[:, sl], start=False, stop=True)
                nc.tensor.matmul(ai_ps[:, :n], dft_c[:, 0:28], wt_i[:, sl], start=True, stop=False)
                nc.tensor.matmul(ai_ps[:, :n], dft_c[:, 28:56], wt_r[:, sl], start=False, stop=True)
                nc.scalar.copy(ar[:, sl], ar_ps[:, :n])
                nc.scalar.copy(ai[:, sl], ai_ps[:, :n])

            at_r = fft.tile([NU, CG * RI], F32R, tag='fb1')
            at_i = fft.tile([NU, CG * RI], F32R, tag='fb2')
            for src_t, dst in ((ar, at_r), (ai, at_i)):
                for half in range(2):
                    tp3 = pfft.tile([NU, 16 * RI], F32, tag='fps')
                    for k in range(16):
                        ci = half * 16 + k
                        nc.tensor.transpose(tp3[:, k * RI:(k + 1) * RI],
                                            src_t[:, ci * NU:(ci + 1) * NU],
                                            ident_f[0:RI, 0:RI])
                    nc.scalar.copy(
                        dst[:].rearrange("p (y c) -> p c y", c=CG)[:, half * 16:(half + 1) * 16, :],
                        tp3[:].rearrange("p (c y) -> p c y", c=16))

            lp_sb = fft.tile([96, CG * RI], F32R, tag='ztlp')
            for half in range(2):
                lp_ps = pfft.tile([96, 448], F32, tag='fps')
                sl = slice(half * 448, (half + 1) * 448)
                nc.tensor.matmul(lp_ps[:], dft_d[:, 0:96], at_r[:, sl], start=True, stop=False)
                nc.tensor.matmul(lp_ps[:], dft_d[:, 96:192], at_i[:, sl], start=False, stop=True)
                nc.scalar.copy(lp_sb[:, sl], lp_ps[:])

            for y4 in range(RI // 4):
                # two [96, 64] transposes cover 4 rows: psum partitions = (y-parity, c)
                t4 = pfft.tile([2 * CG, 2 * 96], F32R, tag='fps')
                y0 = y4 * 4
                nc.tensor.transpose(t4[:, 0:96],
                                    lp_sb[:, y0 * CG:(y0 + 2) * CG], ident[:, :])
                nc.tensor.transpose(t4[:, 96:192],
                                    lp_sb[:, (y0 + 2) * CG:(y0 + 4) * CG], ident[:, :])
                for par in range(2):
                    # psum rows [par*32:(par+1)*32] hold hp rows y0+par and y0+2+par
                    nc.vector.scalar_tensor_tensor(
                        _hp_rows2(hp, c0, y0 + par),
                        _fp_rows2(feats_pad, c0, y0 + par),
                        1.0,
                        t4[par * CG:(par + 1) * CG, :].rearrange("p (y w) -> p y w", y=2),
                        ALU.mult, ALU.subtract)

        def emit_fuzzy(e):
            pwz_t0 = wpool.tile([CH, CH], F32, tag='pwzt')
            nc.scalar.dma_start(pwz_t0[:], I['pwz'][e])
            pwz_t = wpool.tile([CH, CH], F32R, tag='pwztr')
            (nc.scalar.copy if e == 0 else nc.gpsimd.tensor_copy)(pwz_t[:], pwz_t0[:])
            xp = gx.tile([CH, FI], F32, tag='gx')
            XCS = [512, 512, 512, 512, 360, 346]
            XCO = [0, 512, 1024, 1536, 2048, 2398]
            for s in range(6):
                x0 = XCO[s]
                n = XCS[s]
                fpr = ctp.tile([CH, 512], F32R, tag='fpr', bufs=1)
                nc.gpsimd.tensor_copy(fpr[:, :n], feats_pad[:, x0:x0 + n])
                xps = pconv.tile([CH, 512], F32, tag='cps')
                nc.tensor.matmul(xps[:, :n], pwz_t[:], fpr[:, :n], start=True, stop=True)
                nc.vector.scalar_tensor_tensor(xp[:, x0:x0 + n], xps[:, :n],
                                               bvecs[:, 16 + e:17 + e],
                                               fmask_b[:, x0:x0 + n],
                                               ALU.add, ALU.mult)
            gt = gx.tile([CH, FI], F32, tag='gx')
            nc.scalar.activation(gt[:], lt[:], AF.Exp, scale=bvecs[:, 20 + e:21 + e])
            fz = fzp.tile([CH, CT], F32R, tag='fzt')
            nc.vector.memset(fz[:, 0:1].bitcast(mybir.dt.uint32), 0)
            nc.vector.memset(fz[:, CT - 1:CT].bitcast(mybir.dt.uint32), 0)
            nc.vector.tensor_tensor(fz[:, 1:1 + FI], gt[:], xp[:], ALU.mult)
            wz_t0 = wpool.tile([CH, 9 * CH], F32, tag='wct')
            nc.scalar.dma_start(wz_t0[:], I['wz'][e])
            wz_t = wpool.tile([CH, 9 * CH], F32R, tag='wctr')
            (nc.scalar.copy if e == 0 else nc.gpsimd.tensor_copy)(wz_t[:], wz_t0[:])
            conv3x3(fz, wz_t, bvecs[:, 8 + e:9 + e], bvecs[:, 12 + e:13 + e], first=(e == 0))


        for i in range(4):
            emit_fft_group(i)
            emit_fuzzy(i)

        for e in range(4):
            wf_t0 = wpool.tile([CH, 9 * CH], F32, tag='wct')
            nc.scalar.dma_start(wf_t0[:], I['wf'][e])
            wf_t = wpool.tile([CH, 9 * CH], F32R, tag='wctr')
            nc.gpsimd.tensor_copy(wf_t[:], wf_t0[:])
            conv3x3(hp, wf_t, bvecs[:, 0 + e:1 + e], bvecs[:, 4 + e:5 + e], first=False)

        # ================= head =================
        for s in range(6):
            n = CS[s]
            nc.vector.tensor_tensor(acc[:, CO[s]:CO[s] + n], acc[:, CO[s]:CO[s] + n],
                                    feats_pad[:, WP + CO[s]:WP + CO[s] + n], ALU.add)

        o1 = gx.tile([1, FO], F32, tag='gx', name='o1')
        for s in range(6):
            n = CS[s]
            cps = pmisc.tile([1, 512], F32, tag='mps')
            nc.tensor.matmul(cps[:, :n], cow[:], acc[:, CO[s]:CO[s] + n], start=True, stop=True)
            nc.scalar.activation(o1[:, CO[s]:CO[s] + n], cps[:, :n], AF.Identity,
                                 bias=bvecs[0:1, 24:25], scale=1.0)
            nc.sync.dma_start(scratch.rearrange("a b -> (a b)")[CO[s]:CO[s] + n],
                              o1[:, CO[s]:CO[s] + n])

        r_in = small.tile([RO, WP], F32, tag='rin', bufs=1)
        nc.sync.dma_start(r_in[0:13, :], scratch[0:13, :])
        nc.sync.dma_start(r_in[13:26, :], scratch[13:26, :])

        up1 = pmisc.tile([96, 96], F32, tag='mps')
        nc.tensor.matmul(up1[:], r_in[:, 1:97], uy[:], start=True, stop=True)
        rt2 = small.tile([96, 96], F32, tag='rt2', bufs=1)
        nc.vector.tensor_copy(rt2[:], up1[:])
        up2 = pmisc.tile([96, 384], F32, tag='mps')
        nc.tensor.matmul(up2[:], rt2[:], ux[:], start=True, stop=True)
        oup = small.tile([96, 384], F32, tag='oup', bufs=1)
        nc.vector.tensor_copy(oup[:], up2[:])
        nc.sync.dma_start(o_up, oup[:])

    nc.compile()
    return nc


# ============================================================
_NC_CACHE = {}

def _get_nc():
    if 'nc' not in _NC_CACHE:
        _NC_CACHE['nc'] = build_nc()
    return _NC_CACHE['nc']


def _host_router(rts_b, w, b, u):
    pooled = rts_b.reshape(256, -1).astype(np.float64).mean(axis=1)
    logits = w.astype(np.float64) @ pooled + b
    mu = logits.mean()
    sd = logits.std(ddof=1)
    z = (logits - mu) / (sd + 1e-6) - np.log(-np.log(u + 1e-9) + 1e-9)
    z = z - z.max()
    p = np.exp(z)
    return p / p.sum()


def _fold_host(inputs):
    f = {}
    s_f = inputs['fg_g'] / np.sqrt(inputs['fg_v'] + BN_EPS)
    Wf = np.einsum('eoikl,eic->eockl', inputs['fg_cw'], inputs['fg_pw'])
    Wf = Wf * s_f[:, :, None, None, None]
    bf = (inputs['fg_cb'] - inputs['fg_m']) * s_f + inputs['fg_beta']
    f['wf'] = np.ascontiguousarray(Wf.transpose(0, 2, 3, 4, 1).reshape(E, CH, 9 * CH)).astype(np.float32)
    s_z = inputs['iz_g'] / np.sqrt(inputs['iz_v'] + BN_EPS)
    Wz = inputs['iz_cw'] * s_z[:, :, None, None, None]
    f['wz'] = np.ascontiguousarray(Wz.transpose(0, 2, 3, 4, 1).reshape(E, CH, 9 * CH)).astype(np.float32)
    bz = (inputs['iz_cb'] - inputs['iz_m']) * s_z + inputs['iz_beta']
    a = inputs['iz_a'][:, None]
    f['pwz'] = np.ascontiguousarray((inputs['iz_pw'] * a[:, :, None]).transpose(0, 2, 1)).astype(np.float32)
    pbz = inputs['iz_pb'] * a
    rf = inputs['rf_w'].reshape(E, 2, CH).transpose(2, 1, 0)
    rz = inputs['rz_w'].reshape(E, 2, CH).transpose(2, 1, 0)
    f['rww'] = np.concatenate([rf.reshape(CH, 10), rz.reshape(CH, 10)], axis=1).astype(np.float32)
    f['rbias'] = np.concatenate([inputs['rf_b'], inputs['rz_b']])[None].astype(np.float32)
    f['bf'] = bf
    f['bz'] = bz
    f['pbz'] = pbz
    f['mq'] = -np.exp(inputs['iz_p_raw'])
    f['cob'] = inputs['co_b'][0]
    f['cow'] = np.ascontiguousarray(inputs['co_w'].T).astype(np.float32)
    # per-b routing (host mirror; device still computes gate outputs itself)
    f['sel'] = {}
    for b in range(B):
        pf = _host_router(inputs['rts'][b], inputs['rf_w'], inputs['rf_b'], inputs['u_fourier'][b])
        pz = _host_router(inputs['rts'][b], inputs['rz_w'], inputs['rz_b'], inputs['u_fuzzy'][b])
        sf = [e for e in range(E) if e != int(np.argmin(pf))]
        sz = [e for e in range(E) if e != int(np.argmin(pz))]
        f['sel'][b] = (sf, pf, sz, pz)
    return f


def _core_inputs(inputs, f, core):
    b, q = core // 4, core % 4
    ps = PS[q]
    sf, pf, sz, pz = f['sel'][b]
    bv = np.zeros((CH, 25), np.float32)
    bv[:, 0:4] = np.float32(pf[sf])[None, :]
    bv[:, 4:8] = (pf[sf][None, :] * f['bf'][sf].T).astype(np.float32)
    bv[:, 8:12] = np.float32(pz[sz])[None, :]
    bv[:, 12:16] = (pz[sz][None, :] * f['bz'][sz].T).astype(np.float32)
    bv[:, 16:20] = f['pbz'][sz].T
    bv[:, 20:24] = f['mq'][None, sz]
    bv[:, 24] = f['cob']
    feats = inputs['feats'][b]
    fp = np.zeros((CH, RI, WP), np.float32)
    r0, r1 = ps - 1, ps + RI - 1
    sr0, sr1 = max(r0, 0), min(r1, H)
    fp[:, sr0 - r0:sr1 - r0, 1:97] = feats[:, sr0:sr1, :]
    d = {
        'feats_pad': fp.reshape(CH, FI),
        'feats_fft': feats.transpose(1, 0, 2).reshape(H, CH * W),
        'fmask': _percore_fmask(ps),
        'rts_r': inputs['rts'][b].reshape(2, CH, 576).transpose(1, 0, 2).reshape(CH, 2 * 576),
        'rww': f['rww'], 'rbias': f['rbias'],
        'uu': -np.log(-np.log(np.concatenate([inputs['u_fourier'][b], inputs['u_fuzzy'][b]])[None] + 1e-9) + 1e-9),
        'dft_a': DFT_A, 'dft_b': DFT_B, 'dft_c': _percore_dftc(ps), 'dft_d': DFT_D,
        'maskrep': MASKREP, 'ident': IDENT96,
        'wf': f['wf'][sf], 'wz': f['wz'][sz], 'pwz': f['pwz'][sz],
        'bvecs': bv, 'cow': f['cow'],
        'uy': _percore_uy(q), 'ux': UX_T,
    }
    return {k: np.ascontiguousarray(v, dtype=np.float32) for k, v in d.items()}


def make_in_maps(inputs):
    inputs = {k: np.asarray(v, dtype=np.float32) for k, v in inputs.items()}
    f = _fold_host(inputs)
    return [_core_inputs(inputs, f, c) for c in range(8)]


def assemble(results):
    out = np.zeros((B, 1, 4 * H, 4 * W), np.float32)
    gf = np.zeros((B, E), np.float32)
    gz = np.zeros((B, E), np.float32)
    for c in range(8):
        b, q = c // 4, c % 4
        out[b, 0, 96 * q:96 * (q + 1), :] = results[c]['o_up']
        if q == 0:
            gf[b] = results[c]['o_gf'][0]
            gz[b] = results[c]['o_gz'][0]
    return out, gf, gz


class _Exec:
    """Cached 8-core PJRT executor (jit built once, reused across calls)."""

    def __init__(self, nc):
        import jax
        from jax.sharding import Mesh, PartitionSpec
        from jax.experimental.shard_map import shard_map
        from concourse import mybir as _mb
        from concourse.bass2jax import (_bass_exec_p, partition_id_tensor,
                                        install_neuronx_cc_hook)
        install_neuronx_cc_hook()
        self.jax = jax
        in_names, out_names, out_avals, zero_outs = [], [], [], []
        pname = nc.partition_id_tensor.name if nc.partition_id_tensor else None
        for alloc in nc.m.functions[0].allocations:
            if not isinstance(alloc, _mb.MemoryLocationSet):
                continue
            name = alloc.memorylocations[0].name
            if alloc.kind == "ExternalInput":
                if name != pname:
                    in_names.append(name)
            elif alloc.kind == "ExternalOutput":
                shape = list(alloc.tensor_shape)
                np_dt = _mb.dt.np(alloc.dtype)
                out_avals.append(jax.core.ShapedArray(shape, np_dt))
                zero_outs.append(np.zeros(shape, np_dt))
                out_names.append(name)
        self.in_names, self.out_names = list(in_names), out_names
        self.zero_outs = zero_outs
        n_params, n_outs = len(in_names), len(out_names)
        all_names = in_names + out_names + ([pname] if pname else [])

        def _body(*args):
            operands = list(args)
            if pname is not None:
                operands.append(partition_id_tensor())
            outs = _bass_exec_p.bind(
                *operands, out_avals=tuple(out_avals), in_names=tuple(all_names),
                out_names=tuple(out_names), lowering_input_output_aliases=(),
                sim_require_finite=True, sim_require_nnan=True, nc=nc)
            return tuple(outs)

        devices = jax.devices()[:8]
        mesh = Mesh(np.asarray(devices), ("core",))
        in_specs = (PartitionSpec("core"),) * (n_params + n_outs)
        out_specs = (PartitionSpec("core"),) * n_outs
        self.sharded = jax.jit(
            shard_map(_body, mesh=mesh, in_specs=in_specs, out_specs=out_specs,
                      check_rep=False),
            donate_argnums=tuple(range(n_params, n_params + n_outs)),
            keep_unused=True)

    def concat_inputs(self, in_maps):
        return [np.concatenate([np.asarray(m[n]) for m in in_maps], axis=0)
                for n in self.in_names]

    def zeros(self):
        return [np.zeros((8 * z.shape[0], *z.shape[1:]), z.dtype) for z in self.zero_outs]

    def run(self, concat_in):
        outs = self.sharded(*concat_in, *self.zeros())
        return outs

    def to_results(self, outs):
        res = []
        for c in range(8):
            d = {}
            for i, n in enumerate(self.out_names):
                a = np.asarray(outs[i])
                per = a.shape[0] // 8
                d[n] = a[c * per:(c + 1) * per]
            res.append(d)
        return res


def _get_exec():
    if 'exec' not in _NC_CACHE:
        _NC_CACHE['exec'] = _Exec(_get_nc())
    return _NC_CACHE['exec']


def kernel(**inputs):
    ex = _get_exec()
    in_maps = make_in_maps(inputs)
    outs = ex.run(ex.concat_inputs(in_maps))
    return assemble(ex.to_results(outs))
